# revision 7
# baseline (speedup 1.0000x reference)
"""GazeLoss Trainium2 kernel.

The reference bilinear-samples a 32x32 grid from each eye's padded bbox in
pred/target and takes mean L1 of the patch difference.  The sampling grid is
separable (x coords depend only on the grid column, y coords only on the grid
row), so sampling is linear: patch = Gy @ img @ Gx with per-batch sparse
interpolation matrices Gy (32x512, two nonzeros per row) and Gx (512x32).
Since sampling is linear, patch(pred) - patch(target) = Gy @ (pred-target) @ Gx.

Only the image rows with nonzero Gy weight matter: <=128 distinct rows per
batch (union over both eyes), which always fit in <=64 two-row windows.  The
device kernel gathers exactly those windows with SWDGE dma_gather (4 KB per
descriptor; indices are data, so one static NEFF serves every input), then
runs the interpolation as two small matmul stages.

Per core (8 batches, data parallel):
  for each batch b:
    Pg/Tg = dma_gather of 64 two-row windows x 3 channels    (12 MiB/core
            total vs 48 MiB for a full read)
    D     = Pg - Tg                                     (DVE, f32 -> bf16)
    per channel: FT[x,j] = sum_p D[p,x] Gw[p,j]         (PE, 8 matmuls, K=64)
                 O[j,n]  = sum_x FT[x,j] Gx[x,n]        (PE, 4 matmuls)
                 acc[:,img] = rowsum |O| of the two diagonal 32x32 eye blocks
  DMA acc (64, 24) -> DRAM; host sums partials and normalizes.

Gather layout: idx j = c*64 + k -> window k of channel c; out slot
[p=j%128, i=j//128], so channel regions are c0=[p 0:64, slot 0],
c1=[p 64:128, slot 0], c2=[p 0:64, slot 1].  Window weights Gw are host-built
from landmarks only (the grid is constant wrt pred/target, mirroring the
reference's stop_gradient).
"""

import dataclasses

import numpy as np
import ml_dtypes

EYE_SIZE = 32
PAD = 0.3
LEFT_IDX = np.arange(36, 42)
RIGHT_IDX = np.arange(42, 48)
B, C, H, W = 64, 3, 512, 512
S = EYE_SIZE
N_CORES = 8
BPC = B // N_CORES  # batches per core
N_IMG = BPC * C  # images per core
NWIN = 64  # two-row gather windows per (batch, channel)
NIDX = C * NWIN  # gather descriptors per (batch, tensor)
ELEM = 2 * W  # two image rows per gathered element

_COMPILED = None


# ---------------------------------------------------------------------------
# Host-side grid math (pure f32, mirrors the jax reference)
# ---------------------------------------------------------------------------

def _eye_bbox(lm, idx):
    pts = lm[:, idx, :]
    x_min = pts[:, :, 0].min(axis=1)
    x_max = pts[:, :, 0].max(axis=1)
    y_min = pts[:, :, 1].min(axis=1)
    y_max = pts[:, :, 1].max(axis=1)
    w = x_max - x_min
    h = y_max - y_min
    return (x_min - w * PAD, y_min - h * PAD, x_max + w * PAD, y_max + h * PAD)


def _grid_1d(x1, y1, x2, y2):
    # separable sample coords: px (B,S) per grid column, py (B,S) per grid row
    bx1 = np.clip(x1, 0.0, W - 1.0).astype(np.float32)
    by1 = np.clip(y1, 0.0, H - 1.0).astype(np.float32)
    bx2 = np.clip(x2, 0.0, W - 1.0).astype(np.float32)
    by2 = np.clip(y2, 0.0, H - 1.0).astype(np.float32)
    degenerate = (bx2 - bx1 < 1.0) | (by2 - by1 < 1.0)
    xn0 = bx1 / (W - 1) * np.float32(2.0) - np.float32(1.0)
    xn1 = bx2 / (W - 1) * np.float32(2.0) - np.float32(1.0)
    yn0 = by1 / (H - 1) * np.float32(2.0) - np.float32(1.0)
    yn1 = by2 / (H - 1) * np.float32(2.0) - np.float32(1.0)
    t = np.arange(S, dtype=np.float32) / np.float32(S - 1)
    xs = xn0[:, None] + (xn1 - xn0)[:, None] * t
    ys = yn0[:, None] + (yn1 - yn0)[:, None] * t
    xs[degenerate] = 0.0
    ys[degenerate] = 0.0
    px = np.clip((xs + np.float32(1.0)) * np.float32(0.5) * (W - 1), 0.0, W - 1.0)
    py = np.clip((ys + np.float32(1.0)) * np.float32(0.5) * (H - 1), 0.0, H - 1.0)
    return px.astype(np.float32), py.astype(np.float32)


def _interp_pairs(coord):
    # coord (B,S) -> i0, i1 (int), w (f32): value = (1-w)*row[i0] + w*row[i1]
    c0 = np.floor(coord)
    w = (coord - c0).astype(np.float32)
    i0 = np.clip(c0, 0, W - 1).astype(np.int64)
    i1 = np.clip(c0 + 1, 0, W - 1).astype(np.int64)
    return i0, i1, w


def _build_host_tables(landmarks):
    """Per batch: window weights Gw (128, 2, 64), full Gx (512, 64), gather
    idx table (128, NIDX//16) int16."""
    eyes = []
    for idx in (LEFT_IDX, RIGHT_IDX):
        x1, y1, x2, y2 = _eye_bbox(landmarks, idx)
        px, py = _grid_1d(x1, y1, x2, y2)
        eyes.append((_interp_pairs(px), _interp_pairs(py)))

    # dense per-row Gy / per-col Gx weights
    Gy = np.zeros((B, H, 2 * S), dtype=np.float32)
    Gx = np.zeros((B, W, 2 * S), dtype=np.float32)
    for e, ((x0, x1, wx), (y0, y1, wy)) in enumerate(eyes):
        b_idx = np.arange(B)[:, None]
        j = (np.arange(S) + e * S)[None, :]
        np.add.at(Gy, (b_idx, y0, j), np.float32(1.0) - wy)
        np.add.at(Gy, (b_idx, y1, j), wy)
        np.add.at(Gx, (b_idx, x0, j), np.float32(1.0) - wx)
        np.add.at(Gx, (b_idx, x1, j), wx)

    Gw = np.zeros((B, 128, 2, 2 * S), dtype=np.float32)
    idx_tab = np.zeros((B, 128, NIDX // 16), dtype=np.int16)
    for b in range(B):
        rows = set()
        for (_, (y0, y1, _)) in eyes:
            rows.update(y0[b].tolist())
            rows.update(y1[b].tolist())
        rows = sorted(rows)
        # greedy two-row windows covering all needed rows; each covered row is
        # attributed to exactly one (window, half) so weights are not double
        # counted when a clamped window overlaps the previous one
        wins = []
        attrib = {}
        i = 0
        nb = len(rows)
        while i < nb:
            wstart = min(rows[i], H - 2)
            k = len(wins)
            wins.append(wstart)
            for h in (0, 1):
                r = wstart + h
                if r not in attrib:
                    attrib[r] = (k, h)
            while i < nb and rows[i] <= wstart + 1:
                i += 1
        n_win = len(wins)
        assert n_win <= NWIN, n_win
        win_arr = np.zeros(NWIN, dtype=np.int64)
        win_arr[:n_win] = wins
        for r, (k, h) in attrib.items():
            Gw[b, k, h] = Gy[b, r]
            Gw[b, 64 + k, h] = Gy[b, r]
        # idx j = c*64 + k -> element = two rows starting at (c, win[k])
        flat = (np.arange(C)[:, None] * H + win_arr[None, :]).reshape(-1)
        wrapped = flat.reshape(NIDX // 16, 16).T.astype(np.int16)
        idx_tab[b] = np.tile(wrapped, (8, 1))
    return Gw, Gx, idx_tab


# ---------------------------------------------------------------------------
# Device kernel
# ---------------------------------------------------------------------------

def _build_nc():
    import concourse.mybir as mybir
    import concourse.tile as tile
    from concourse import bacc
    from concourse import library_config

    f32 = mybir.dt.float32
    bf16 = mybir.dt.bfloat16
    i16 = mybir.dt.int16

    nc = bacc.Bacc("TRN2", target_bir_lowering=False, debug=False,
                   num_devices=N_CORES)
    pred = nc.dram_tensor("pred", (BPC, C, H, W), f32, kind="ExternalInput")
    target = nc.dram_tensor("target", (BPC, C, H, W), f32, kind="ExternalInput")
    gw = nc.dram_tensor("gw", (BPC, 128, 2, 2 * S), bf16, kind="ExternalInput")
    # gx packed (row_in_chunk, chunk, batch, 64): sbuf free = chunk, b*64+j
    gx = nc.dram_tensor("gx", (128, 4, BPC, 2 * S), bf16, kind="ExternalInput")
    idx = nc.dram_tensor("idx", (BPC, 128, NIDX // 16), i16, kind="ExternalInput")
    out = nc.dram_tensor("o", (2 * S, N_IMG), f32, kind="ExternalOutput")

    def rows_view(img_ap):
        # (C, H, W) -> overlapping two-row windows: row i covers rows [i, i+2)
        flat = img_ap.rearrange("c h w -> (c h) w")
        return dataclasses.replace(flat, ap=[[W, C * H - 1], [1, ELEM]])

    with tile.TileContext(nc) as tc:
        with (
            tc.tile_pool(name="consts", bufs=1) as cpool,
            tc.tile_pool(name="gat", bufs=4) as gpool,
            tc.tile_pool(name="dtile", bufs=2) as d_pool,
            tc.tile_pool(name="ft", bufs=2) as ft_pool,
            tc.tile_pool(name="ps_ft", bufs=2, space="PSUM") as psf_pool,
            tc.tile_pool(name="ps_o", bufs=2, space="PSUM") as pso_pool,
        ):
            nc.gpsimd.load_library(library_config.mlp)

            idx_s = cpool.tile([128, BPC, NIDX // 16], i16, name="idx_s")
            gw_s = cpool.tile([128, BPC, 2, 2 * S], bf16, name="gw_s")
            gx_s = cpool.tile([128, 4, BPC * 2 * S], bf16, name="gx_s")
            acc = cpool.tile([2 * S, N_IMG], f32, name="acc")
            nc.sync.dma_start(out=idx_s[:], in_=idx.ap().rearrange("b p s -> p b s"))
            nc.sync.dma_start(out=gw_s[:], in_=gw.ap().rearrange("b p h r -> p b h r"))
            nc.sync.dma_start(out=gx_s[:], in_=gx.ap().rearrange("p c b r -> p c (b r)"))

            for b in range(BPC):
                pg = gpool.tile([128, 2, ELEM], f32, tag="pg")
                tg = gpool.tile([128, 2, ELEM], f32, tag="tg")
                nc.gpsimd.dma_gather(
                    pg[:], rows_view(pred.ap()[b]), idx_s[:, b, :],
                    NIDX, NIDX, ELEM, elem_step=W,
                )
                nc.gpsimd.dma_gather(
                    tg[:], rows_view(target.ap()[b]), idx_s[:, b, :],
                    NIDX, NIDX, ELEM, elem_step=W,
                )
                # slots [0:64, 1] hold channel 2; [64:128, 1] is never gathered,
                # so subtract only the written regions
                d = d_pool.tile([128, 2, ELEM], bf16, tag="d")
                nc.vector.tensor_sub(d[:, 0, :], pg[:, 0, :], tg[:, 0, :])
                nc.vector.tensor_sub(d[0:64, 1, :], pg[0:64, 1, :], tg[0:64, 1, :])

                for ch in range(C):
                    j = b * C + ch
                    pc0 = 64 * (ch % 2)
                    sc = ch // 2
                    # FT[x, j] = sum_{k,h} D[pc0+k, sc, h*W+x] * Gw[pc0+k, h, j]
                    ftp = psf_pool.tile([128, 4 * 2 * S], f32, tag="ftp")
                    for xch in range(4):
                        for h in range(2):
                            nc.tensor.matmul(
                                ftp[:, xch * 64:(xch + 1) * 64],
                                d[pc0:pc0 + 64, sc,
                                  h * W + xch * 128: h * W + (xch + 1) * 128],
                                gw_s[pc0:pc0 + 64, b, h, :],
                                start=(h == 0), stop=(h == 1),
                            )
                    ft = ft_pool.tile([128, 4 * 2 * S], bf16, tag="ft")
                    nc.scalar.copy(ft[:], ftp[:])

                    # O[j, n] = sum_x FT[x, j] * Gx[x, n]
                    op = pso_pool.tile([2 * S, 2 * S], f32, tag="op")
                    for xch in range(4):
                        nc.tensor.matmul(
                            op[:],
                            ft[:, xch * 64:(xch + 1) * 64],
                            gx_s[:, xch, b * 64: b * 64 + 64],
                            start=(xch == 0), stop=(xch == 3),
                        )
                    nc.vector.tensor_reduce(
                        acc[0:S, j:j + 1], op[0:S, 0:S],
                        axis=mybir.AxisListType.X, op=mybir.AluOpType.add,
                        apply_absolute_value=True,
                    )
                    nc.vector.tensor_reduce(
                        acc[S:2 * S, j:j + 1], op[S:2 * S, S:2 * S],
                        axis=mybir.AxisListType.X, op=mybir.AluOpType.add,
                        apply_absolute_value=True,
                    )

            nc.sync.dma_start(out=out.ap()[:, :], in_=acc[:])

    nc.compile()
    return nc


def _get_compiled():
    global _COMPILED
    if _COMPILED is None:
        _COMPILED = _build_nc()
    return _COMPILED


def _run_device(pred, target, landmarks, trace=False):
    from concourse import bass_utils

    pred = np.ascontiguousarray(np.asarray(pred, dtype=np.float32))
    target = np.ascontiguousarray(np.asarray(target, dtype=np.float32))
    landmarks = np.asarray(landmarks, dtype=np.float32)

    Gw, Gx, idx_tab = _build_host_tables(landmarks)
    gw_bf = Gw.astype(ml_dtypes.bfloat16)
    # (B, 512, 64) -> (128, 4, B, 64) bf16
    gx_packed = np.ascontiguousarray(
        Gx.reshape(B, 4, 128, 2 * S).transpose(2, 1, 0, 3)
    ).astype(ml_dtypes.bfloat16)

    in_maps = []
    for i in range(N_CORES):
        lo, hi = i * BPC, (i + 1) * BPC
        in_maps.append({
            "pred": pred[lo:hi],
            "target": target[lo:hi],
            "gw": gw_bf[lo:hi],
            "gx": np.ascontiguousarray(gx_packed[:, :, lo:hi]),
            "idx": idx_tab[lo:hi],
        })

    nc = _get_compiled()
    res = bass_utils.run_bass_kernel_spmd(
        nc, in_maps, core_ids=list(range(N_CORES)), trace=trace
    )
    total = np.float64(0.0)
    for i in range(N_CORES):
        total += res.results[i]["o"].astype(np.float64).sum()
    n = B * C * S * S
    loss = total / n / 2.0
    return np.float32(loss), res


def kernel(pred, target, landmarks):
    loss, _ = _run_device(pred, target, landmarks, trace=False)
    return loss


# revision 8
# speedup vs baseline: 1.2008x; 1.2008x over previous
"""GazeLoss Trainium2 kernel.

The reference bilinear-samples a 32x32 grid from each eye's padded bbox in
pred/target and takes mean L1 of the patch difference.  The sampling grid is
separable (x coords depend only on the grid column, y coords only on the grid
row), so sampling is linear: patch = Gy @ img @ Gx with per-batch sparse
interpolation matrices Gy (32x512, two nonzeros per row) and Gx (512x32).
Since sampling is linear, patch(pred) - patch(target) = Gy @ (pred-target) @ Gx.

Only the image rows with nonzero Gy weight matter: <=128 distinct rows per
batch (union over both eyes), which always fit in <=64 two-row windows.  The
device kernel gathers exactly those windows with SWDGE dma_gather (4 KB per
descriptor; indices are data, so one static NEFF serves every input), then
runs the interpolation as two small matmul stages.

Per core (8 batches, data parallel):
  for each batch b:
    Pg/Tg = dma_gather of 64 two-row windows x 3 channels    (12 MiB/core
            total vs 48 MiB for a full read)
    D     = Pg - Tg                                     (DVE, f32 -> bf16)
    per channel: FT[x,j] = sum_p D[p,x] Gw[p,j]         (PE, 8 matmuls, K=64)
                 O[j,n]  = sum_x FT[x,j] Gx[x,n]        (PE, 4 matmuls)
                 acc[:,img] = rowsum |O| of the two diagonal 32x32 eye blocks
  DMA acc (64, 24) -> DRAM; host sums partials and normalizes.

Gather layout: idx j = c*64 + k -> window k of channel c; out slot
[p=j%128, i=j//128], so channel regions are c0=[p 0:64, slot 0],
c1=[p 64:128, slot 0], c2=[p 0:64, slot 1].  Window weights Gw are host-built
from landmarks only (the grid is constant wrt pred/target, mirroring the
reference's stop_gradient).
"""

import dataclasses

import numpy as np
import ml_dtypes

EYE_SIZE = 32
PAD = 0.3
LEFT_IDX = np.arange(36, 42)
RIGHT_IDX = np.arange(42, 48)
B, C, H, W = 64, 3, 512, 512
S = EYE_SIZE
N_CORES = 8
BPC = B // N_CORES  # batches per core
N_IMG = BPC * C  # images per core
NWIN = 64  # two-row gather windows per (batch, channel)
NIDX = 2 * C * NWIN  # gather descriptors per (batch-pair, tensor)
ELEM = 2 * W  # two image rows per gathered element

_COMPILED = None


# ---------------------------------------------------------------------------
# Host-side grid math (pure f32, mirrors the jax reference)
# ---------------------------------------------------------------------------

def _eye_bbox(lm, idx):
    pts = lm[:, idx, :]
    x_min = pts[:, :, 0].min(axis=1)
    x_max = pts[:, :, 0].max(axis=1)
    y_min = pts[:, :, 1].min(axis=1)
    y_max = pts[:, :, 1].max(axis=1)
    w = x_max - x_min
    h = y_max - y_min
    return (x_min - w * PAD, y_min - h * PAD, x_max + w * PAD, y_max + h * PAD)


def _grid_1d(x1, y1, x2, y2):
    # separable sample coords: px (B,S) per grid column, py (B,S) per grid row
    bx1 = np.clip(x1, 0.0, W - 1.0).astype(np.float32)
    by1 = np.clip(y1, 0.0, H - 1.0).astype(np.float32)
    bx2 = np.clip(x2, 0.0, W - 1.0).astype(np.float32)
    by2 = np.clip(y2, 0.0, H - 1.0).astype(np.float32)
    degenerate = (bx2 - bx1 < 1.0) | (by2 - by1 < 1.0)
    xn0 = bx1 / (W - 1) * np.float32(2.0) - np.float32(1.0)
    xn1 = bx2 / (W - 1) * np.float32(2.0) - np.float32(1.0)
    yn0 = by1 / (H - 1) * np.float32(2.0) - np.float32(1.0)
    yn1 = by2 / (H - 1) * np.float32(2.0) - np.float32(1.0)
    t = np.arange(S, dtype=np.float32) / np.float32(S - 1)
    xs = xn0[:, None] + (xn1 - xn0)[:, None] * t
    ys = yn0[:, None] + (yn1 - yn0)[:, None] * t
    xs[degenerate] = 0.0
    ys[degenerate] = 0.0
    px = np.clip((xs + np.float32(1.0)) * np.float32(0.5) * (W - 1), 0.0, W - 1.0)
    py = np.clip((ys + np.float32(1.0)) * np.float32(0.5) * (H - 1), 0.0, H - 1.0)
    return px.astype(np.float32), py.astype(np.float32)


def _interp_pairs(coord):
    # coord (B,S) -> i0, i1 (int), w (f32): value = (1-w)*row[i0] + w*row[i1]
    c0 = np.floor(coord)
    w = (coord - c0).astype(np.float32)
    i0 = np.clip(c0, 0, W - 1).astype(np.int64)
    i1 = np.clip(c0 + 1, 0, W - 1).astype(np.int64)
    return i0, i1, w


def _build_host_tables(landmarks):
    """Per batch: window weights Gw (128, 2, 64), full Gx (512, 64), gather
    idx table (128, NIDX//16) int16."""
    eyes = []
    for idx in (LEFT_IDX, RIGHT_IDX):
        x1, y1, x2, y2 = _eye_bbox(landmarks, idx)
        px, py = _grid_1d(x1, y1, x2, y2)
        eyes.append((_interp_pairs(px), _interp_pairs(py)))

    # dense per-row Gy / per-col Gx weights
    Gy = np.zeros((B, H, 2 * S), dtype=np.float32)
    Gx = np.zeros((B, W, 2 * S), dtype=np.float32)
    for e, ((x0, x1, wx), (y0, y1, wy)) in enumerate(eyes):
        b_idx = np.arange(B)[:, None]
        j = (np.arange(S) + e * S)[None, :]
        np.add.at(Gy, (b_idx, y0, j), np.float32(1.0) - wy)
        np.add.at(Gy, (b_idx, y1, j), wy)
        np.add.at(Gx, (b_idx, x0, j), np.float32(1.0) - wx)
        np.add.at(Gx, (b_idx, x1, j), wx)

    Gw = np.zeros((B, 128, 2, 2 * S), dtype=np.float32)
    win_all = np.zeros((B, NWIN), dtype=np.int64)
    for b in range(B):
        rows = set()
        for (_, (y0, y1, _)) in eyes:
            rows.update(y0[b].tolist())
            rows.update(y1[b].tolist())
        rows = sorted(rows)
        # greedy two-row windows covering all needed rows; each covered row is
        # attributed to exactly one (window, half) so weights are not double
        # counted when a clamped window overlaps the previous one
        wins = []
        attrib = {}
        i = 0
        nb = len(rows)
        while i < nb:
            wstart = min(rows[i], H - 2)
            k = len(wins)
            wins.append(wstart)
            for h in (0, 1):
                r = wstart + h
                if r not in attrib:
                    attrib[r] = (k, h)
            while i < nb and rows[i] <= wstart + 1:
                i += 1
        n_win = len(wins)
        assert n_win <= NWIN, n_win
        win_all[b, :n_win] = wins
        for r, (k, h) in attrib.items():
            Gw[b, k, h] = Gy[b, r]
            Gw[b, 64 + k, h] = Gy[b, r]
    # idx j = b2*192 + c*64 + k -> two rows starting at (b2, c, win[b2][k])
    idx_tab = np.zeros((B // 2, 128, NIDX // 16), dtype=np.int16)
    for g in range(B // 2):
        flat = (np.arange(2)[:, None, None] * (C * H)
                + np.arange(C)[None, :, None] * H
                + win_all[2 * g:2 * g + 2][:, None, :]).reshape(-1)
        wrapped = flat.reshape(NIDX // 16, 16).T.astype(np.int16)
        idx_tab[g] = np.tile(wrapped, (8, 1))
    return Gw, Gx, idx_tab


# ---------------------------------------------------------------------------
# Device kernel
# ---------------------------------------------------------------------------

def _build_nc():
    import concourse.mybir as mybir
    import concourse.tile as tile
    from concourse import bacc
    from concourse import library_config

    f32 = mybir.dt.float32
    bf16 = mybir.dt.bfloat16
    i16 = mybir.dt.int16

    nc = bacc.Bacc("TRN2", target_bir_lowering=False, debug=False,
                   num_devices=N_CORES)
    pred = nc.dram_tensor("pred", (BPC, C, H, W), f32, kind="ExternalInput")
    target = nc.dram_tensor("target", (BPC, C, H, W), f32, kind="ExternalInput")
    gw = nc.dram_tensor("gw", (BPC, 128, 2, 2 * S), bf16, kind="ExternalInput")
    # gx packed (row_in_chunk, chunk, batch, 64): sbuf free = chunk, b*64+j
    gx = nc.dram_tensor("gx", (128, 4, BPC, 2 * S), bf16, kind="ExternalInput")
    idx = nc.dram_tensor("idx", (BPC // 2, 128, NIDX // 16), i16, kind="ExternalInput")
    out = nc.dram_tensor("o", (2 * S, N_IMG), f32, kind="ExternalOutput")

    def rows_view(pair_ap):
        # (2, C, H, W) -> overlapping two-row windows: row i covers [i, i+2)
        flat = pair_ap.rearrange("b c h w -> (b c h) w")
        return dataclasses.replace(flat, ap=[[W, 2 * C * H - 1], [1, ELEM]])

    with tile.TileContext(nc) as tc:
        with (
            tc.tile_pool(name="consts", bufs=1) as cpool,
            tc.tile_pool(name="gat", bufs=3) as gpool,
            tc.tile_pool(name="dtile", bufs=2) as d_pool,
            tc.tile_pool(name="ft", bufs=2) as ft_pool,
            tc.tile_pool(name="ps_ft", bufs=3, space="PSUM") as psf_pool,
            tc.tile_pool(name="ps_o", bufs=3, space="PSUM") as pso_pool,
        ):
            nc.gpsimd.load_library(library_config.mlp)

            idx_s = cpool.tile([128, BPC // 2, NIDX // 16], i16, name="idx_s")
            gw_s = cpool.tile([128, BPC, 2, 2 * S], bf16, name="gw_s")
            gx_s = cpool.tile([128, 4, BPC * 2 * S], bf16, name="gx_s")
            acc = cpool.tile([2 * S, N_IMG], f32, name="acc")
            nc.sync.dma_start(out=idx_s[:], in_=idx.ap().rearrange("b p s -> p b s"))
            nc.sync.dma_start(out=gw_s[:], in_=gw.ap().rearrange("b p h r -> p b h r"))
            nc.sync.dma_start(out=gx_s[:], in_=gx.ap().rearrange("p c b r -> p c (b r)"))

            for g in range(BPC // 2):
                pg = gpool.tile([128, 3, ELEM], f32, tag="pg")
                tg = gpool.tile([128, 3, ELEM], f32, tag="tg")
                nc.gpsimd.dma_gather(
                    pg[:], rows_view(pred.ap()[2 * g:2 * g + 2]), idx_s[:, g, :],
                    NIDX, NIDX, ELEM, elem_step=W,
                )
                nc.gpsimd.dma_gather(
                    tg[:], rows_view(target.ap()[2 * g:2 * g + 2]), idx_s[:, g, :],
                    NIDX, NIDX, ELEM, elem_step=W,
                )
                d = d_pool.tile([128, 3, ELEM], bf16, tag="d")
                nc.vector.tensor_sub(d[:], pg[:], tg[:])

                for bc in range(2 * C):
                    b2, ch = divmod(bc, C)
                    b = 2 * g + b2
                    j = b * C + ch
                    # slot/partition-half region for (b2, ch): j_idx = b2*192+ch*64
                    pc0 = 64 * ((b2 * C + ch) % 2)
                    sc = (b2 * C + ch) // 2
                    # FT[x, j] = sum_{k,h} D[pc0+k, sc, h*W+x] * Gw[pc0+k, h, j]
                    ftp = psf_pool.tile([128, 4 * 2 * S], f32, tag="ftp")
                    for xch in range(4):
                        for h in range(2):
                            nc.tensor.matmul(
                                ftp[:, xch * 64:(xch + 1) * 64],
                                d[pc0:pc0 + 64, sc,
                                  h * W + xch * 128: h * W + (xch + 1) * 128],
                                gw_s[pc0:pc0 + 64, b, h, :],
                                start=(h == 0), stop=(h == 1),
                            )
                    ft = ft_pool.tile([128, 4 * 2 * S], bf16, tag="ft")
                    nc.scalar.copy(ft[:], ftp[:])

                    # O[j, n] = sum_x FT[x, j] * Gx[x, n]
                    op = pso_pool.tile([2 * S, 2 * S], f32, tag="op")
                    for xch in range(4):
                        nc.tensor.matmul(
                            op[:],
                            ft[:, xch * 64:(xch + 1) * 64],
                            gx_s[:, xch, b * 64: b * 64 + 64],
                            start=(xch == 0), stop=(xch == 3),
                        )
                    nc.vector.tensor_reduce(
                        acc[0:S, j:j + 1], op[0:S, 0:S],
                        axis=mybir.AxisListType.X, op=mybir.AluOpType.add,
                        apply_absolute_value=True,
                    )
                    nc.vector.tensor_reduce(
                        acc[S:2 * S, j:j + 1], op[S:2 * S, S:2 * S],
                        axis=mybir.AxisListType.X, op=mybir.AluOpType.add,
                        apply_absolute_value=True,
                    )

            nc.sync.dma_start(out=out.ap()[:, :], in_=acc[:])

    nc.compile()
    return nc


def _get_compiled():
    global _COMPILED
    if _COMPILED is None:
        _COMPILED = _build_nc()
    return _COMPILED


def _run_device(pred, target, landmarks, trace=False):
    from concourse import bass_utils

    pred = np.ascontiguousarray(np.asarray(pred, dtype=np.float32))
    target = np.ascontiguousarray(np.asarray(target, dtype=np.float32))
    landmarks = np.asarray(landmarks, dtype=np.float32)

    Gw, Gx, idx_tab = _build_host_tables(landmarks)
    gw_bf = Gw.astype(ml_dtypes.bfloat16)
    # (B, 512, 64) -> (128, 4, B, 64) bf16
    gx_packed = np.ascontiguousarray(
        Gx.reshape(B, 4, 128, 2 * S).transpose(2, 1, 0, 3)
    ).astype(ml_dtypes.bfloat16)

    in_maps = []
    for i in range(N_CORES):
        lo, hi = i * BPC, (i + 1) * BPC
        in_maps.append({
            "pred": pred[lo:hi],
            "target": target[lo:hi],
            "gw": gw_bf[lo:hi],
            "gx": np.ascontiguousarray(gx_packed[:, :, lo:hi]),
            "idx": idx_tab[lo // 2: hi // 2],
        })

    nc = _get_compiled()
    res = bass_utils.run_bass_kernel_spmd(
        nc, in_maps, core_ids=list(range(N_CORES)), trace=trace
    )
    total = np.float64(0.0)
    for i in range(N_CORES):
        total += res.results[i]["o"].astype(np.float64).sum()
    n = B * C * S * S
    loss = total / n / 2.0
    return np.float32(loss), res


def kernel(pred, target, landmarks):
    loss, _ = _run_device(pred, target, landmarks, trace=False)
    return loss


# revision 9
# speedup vs baseline: 1.2864x; 1.0713x over previous
"""GazeLoss Trainium2 kernel.

The reference bilinear-samples a 32x32 grid from each eye's padded bbox in
pred/target and takes mean L1 of the patch difference.  The sampling grid is
separable (x coords depend only on the grid column, y coords only on the grid
row), so sampling is linear: patch = Gy @ img @ Gx with per-batch sparse
interpolation matrices Gy (32x512, two nonzeros per row) and Gx (512x32).
Since sampling is linear, patch(pred) - patch(target) = Gy @ (pred-target) @ Gx.

Only the image rows with nonzero Gy weight matter: <=128 distinct rows per
batch (union over both eyes), which always fit in <=64 two-row windows.  The
device kernel gathers exactly those windows with SWDGE dma_gather (4 KB per
descriptor; indices are data, so one static NEFF serves every input), then
runs the interpolation as two small matmul stages.

Per core (8 batches, data parallel):
  for each batch b:
    Pg/Tg = dma_gather of 64 two-row windows x 3 channels    (12 MiB/core
            total vs 48 MiB for a full read)
    D     = Pg - Tg                                     (DVE, f32 -> bf16)
    per channel: FT[x,j] = sum_p D[p,x] Gw[p,j]         (PE, 8 matmuls, K=64)
                 O[j,n]  = sum_x FT[x,j] Gx[x,n]        (PE, 4 matmuls)
                 acc[:,img] = rowsum |O| of the two diagonal 32x32 eye blocks
  DMA acc (64, 24) -> DRAM; host sums partials and normalizes.

Gather layout: idx j = c*64 + k -> window k of channel c; out slot
[p=j%128, i=j//128], so channel regions are c0=[p 0:64, slot 0],
c1=[p 64:128, slot 0], c2=[p 0:64, slot 1].  Window weights Gw are host-built
from landmarks only (the grid is constant wrt pred/target, mirroring the
reference's stop_gradient).
"""

import dataclasses

import numpy as np
import ml_dtypes

EYE_SIZE = 32
PAD = 0.3
LEFT_IDX = np.arange(36, 42)
RIGHT_IDX = np.arange(42, 48)
B, C, H, W = 64, 3, 512, 512
S = EYE_SIZE
N_CORES = 8
BPC = B // N_CORES  # batches per core
N_IMG = BPC * C  # images per core
NWIN = 64  # two-row gather windows per (batch, channel)
NIDX = 2 * C * NWIN  # gather descriptors per (batch-pair, tensor)
ELEM = 2 * W  # two image rows per gathered element

_COMPILED = None


# ---------------------------------------------------------------------------
# Host-side grid math (pure f32, mirrors the jax reference)
# ---------------------------------------------------------------------------

def _eye_bbox(lm, idx):
    pts = lm[:, idx, :]
    x_min = pts[:, :, 0].min(axis=1)
    x_max = pts[:, :, 0].max(axis=1)
    y_min = pts[:, :, 1].min(axis=1)
    y_max = pts[:, :, 1].max(axis=1)
    w = x_max - x_min
    h = y_max - y_min
    return (x_min - w * PAD, y_min - h * PAD, x_max + w * PAD, y_max + h * PAD)


def _grid_1d(x1, y1, x2, y2):
    # separable sample coords: px (B,S) per grid column, py (B,S) per grid row
    bx1 = np.clip(x1, 0.0, W - 1.0).astype(np.float32)
    by1 = np.clip(y1, 0.0, H - 1.0).astype(np.float32)
    bx2 = np.clip(x2, 0.0, W - 1.0).astype(np.float32)
    by2 = np.clip(y2, 0.0, H - 1.0).astype(np.float32)
    degenerate = (bx2 - bx1 < 1.0) | (by2 - by1 < 1.0)
    xn0 = bx1 / (W - 1) * np.float32(2.0) - np.float32(1.0)
    xn1 = bx2 / (W - 1) * np.float32(2.0) - np.float32(1.0)
    yn0 = by1 / (H - 1) * np.float32(2.0) - np.float32(1.0)
    yn1 = by2 / (H - 1) * np.float32(2.0) - np.float32(1.0)
    t = np.arange(S, dtype=np.float32) / np.float32(S - 1)
    xs = xn0[:, None] + (xn1 - xn0)[:, None] * t
    ys = yn0[:, None] + (yn1 - yn0)[:, None] * t
    xs[degenerate] = 0.0
    ys[degenerate] = 0.0
    px = np.clip((xs + np.float32(1.0)) * np.float32(0.5) * (W - 1), 0.0, W - 1.0)
    py = np.clip((ys + np.float32(1.0)) * np.float32(0.5) * (H - 1), 0.0, H - 1.0)
    return px.astype(np.float32), py.astype(np.float32)


def _interp_pairs(coord):
    # coord (B,S) -> i0, i1 (int), w (f32): value = (1-w)*row[i0] + w*row[i1]
    c0 = np.floor(coord)
    w = (coord - c0).astype(np.float32)
    i0 = np.clip(c0, 0, W - 1).astype(np.int64)
    i1 = np.clip(c0 + 1, 0, W - 1).astype(np.int64)
    return i0, i1, w


def _build_host_tables(landmarks):
    """Per batch: window weights Gw (128, 2, 64), full Gx (512, 64), gather
    idx table (128, NIDX//16) int16."""
    eyes = []
    for idx in (LEFT_IDX, RIGHT_IDX):
        x1, y1, x2, y2 = _eye_bbox(landmarks, idx)
        px, py = _grid_1d(x1, y1, x2, y2)
        eyes.append((_interp_pairs(px), _interp_pairs(py)))

    # dense per-row Gy / per-col Gx weights
    Gy = np.zeros((B, H, 2 * S), dtype=np.float32)
    Gx = np.zeros((B, W, 2 * S), dtype=np.float32)
    for e, ((x0, x1, wx), (y0, y1, wy)) in enumerate(eyes):
        b_idx = np.arange(B)[:, None]
        j = (np.arange(S) + e * S)[None, :]
        np.add.at(Gy, (b_idx, y0, j), np.float32(1.0) - wy)
        np.add.at(Gy, (b_idx, y1, j), wy)
        np.add.at(Gx, (b_idx, x0, j), np.float32(1.0) - wx)
        np.add.at(Gx, (b_idx, x1, j), wx)

    Gw = np.zeros((B, 128, 2, 2 * S), dtype=np.float32)
    win_all = np.zeros((B, NWIN), dtype=np.int64)
    for b in range(B):
        rows = set()
        for (_, (y0, y1, _)) in eyes:
            rows.update(y0[b].tolist())
            rows.update(y1[b].tolist())
        rows = sorted(rows)
        # greedy two-row windows covering all needed rows; each covered row is
        # attributed to exactly one (window, half) so weights are not double
        # counted when a clamped window overlaps the previous one
        wins = []
        attrib = {}
        i = 0
        nb = len(rows)
        while i < nb:
            wstart = min(rows[i], H - 2)
            k = len(wins)
            wins.append(wstart)
            for h in (0, 1):
                r = wstart + h
                if r not in attrib:
                    attrib[r] = (k, h)
            while i < nb and rows[i] <= wstart + 1:
                i += 1
        n_win = len(wins)
        assert n_win <= NWIN, n_win
        win_all[b, :n_win] = wins
        for r, (k, h) in attrib.items():
            Gw[b, k, h] = Gy[b, r]
            Gw[b, 64 + k, h] = Gy[b, r]
    # idx j = b2*192 + c*64 + k -> two rows starting at (b2, c, win[b2][k])
    idx_tab = np.zeros((B // 2, 128, NIDX // 16), dtype=np.int16)
    for g in range(B // 2):
        flat = (np.arange(2)[:, None, None] * (C * H)
                + np.arange(C)[None, :, None] * H
                + win_all[2 * g:2 * g + 2][:, None, :]).reshape(-1)
        wrapped = flat.reshape(NIDX // 16, 16).T.astype(np.int16)
        idx_tab[g] = np.tile(wrapped, (8, 1))
    # device layout: (128, G*12) with per-core slabs concatenated on the free dim
    return Gw, Gx, idx_tab


# ---------------------------------------------------------------------------
# Device kernel
# ---------------------------------------------------------------------------

def _build_nc():
    import concourse.mybir as mybir
    import concourse.tile as tile
    from concourse import bacc
    from concourse import library_config

    f32 = mybir.dt.float32
    bf16 = mybir.dt.bfloat16
    i16 = mybir.dt.int16

    nc = bacc.Bacc("TRN2", target_bir_lowering=False, debug=False,
                   num_devices=N_CORES)
    pred = nc.dram_tensor("pred", (BPC, C, H, W), f32, kind="ExternalInput")
    target = nc.dram_tensor("target", (BPC, C, H, W), f32, kind="ExternalInput")
    gw = nc.dram_tensor("gw", (BPC, 128, 2, 2 * S), bf16, kind="ExternalInput")
    # gx packed (row_in_chunk, chunk, batch, 64): sbuf free = chunk, b*64+j
    gx = nc.dram_tensor("gx", (128, 4, BPC, 2 * S), bf16, kind="ExternalInput")
    idx = nc.dram_tensor("idx", (128, (BPC // 2) * (NIDX // 16)), i16, kind="ExternalInput")
    out = nc.dram_tensor("o", (2 * S, N_IMG), f32, kind="ExternalOutput")

    def rows_view(pair_ap):
        # (2, C, H, W) -> overlapping two-row windows: row i covers [i, i+2)
        flat = pair_ap.rearrange("b c h w -> (b c h) w")
        return dataclasses.replace(flat, ap=[[W, 2 * C * H - 1], [1, ELEM]])

    with tile.TileContext(nc) as tc:
        with (
            tc.tile_pool(name="consts", bufs=1) as cpool,
            tc.tile_pool(name="gat", bufs=3) as gpool,
            tc.tile_pool(name="dtile", bufs=3) as d_pool,
            tc.tile_pool(name="ft", bufs=3) as ft_pool,
            tc.tile_pool(name="ps_ft", bufs=4, space="PSUM") as psf_pool,
            tc.tile_pool(name="ps_o", bufs=4, space="PSUM") as pso_pool,
        ):
            nc.gpsimd.load_library(library_config.mlp)

            idx_s = cpool.tile([128, BPC // 2, NIDX // 16], i16, name="idx_s")
            gw_s = cpool.tile([128, BPC, 2, 2 * S], bf16, name="gw_s")
            gx_s = cpool.tile([128, 4, BPC * 2 * S], bf16, name="gx_s")
            acc = cpool.tile([2 * S, N_IMG], f32, name="acc")
            nc.sync.dma_start(out=idx_s[:], in_=idx.ap().rearrange("p (b s) -> p b s", s=NIDX // 16))
            nc.scalar.dma_start(out=gw_s[:], in_=gw.ap().rearrange("b p h r -> p b h r"))
            nc.scalar.dma_start(out=gx_s[:], in_=gx.ap().rearrange("p c b r -> p c (b r)"))

            for g in range(BPC // 2):
                pg = gpool.tile([128, 3, ELEM], f32, tag="pg")
                tg = gpool.tile([128, 3, ELEM], f32, tag="tg")
                nc.gpsimd.dma_gather(
                    pg[:], rows_view(pred.ap()[2 * g:2 * g + 2]), idx_s[:, g, :],
                    NIDX, NIDX, ELEM, elem_step=W,
                )
                nc.gpsimd.dma_gather(
                    tg[:], rows_view(target.ap()[2 * g:2 * g + 2]), idx_s[:, g, :],
                    NIDX, NIDX, ELEM, elem_step=W,
                )
                d = d_pool.tile([128, 3, ELEM], bf16, tag="d")
                nc.vector.tensor_sub(d[:], pg[:], tg[:])

                for bc in range(2 * C):
                    b2, ch = divmod(bc, C)
                    b = 2 * g + b2
                    j = b * C + ch
                    # slot/partition-half region for (b2, ch): j_idx = b2*192+ch*64
                    pc0 = 64 * ((b2 * C + ch) % 2)
                    sc = (b2 * C + ch) // 2
                    # FT[x, j] = sum_{k,h} D[pc0+k, sc, h*W+x] * Gw[pc0+k, h, j]
                    ftp = psf_pool.tile([128, 4 * 2 * S], f32, tag="ftp")
                    for xch in range(4):
                        for h in range(2):
                            nc.tensor.matmul(
                                ftp[:, xch * 64:(xch + 1) * 64],
                                d[pc0:pc0 + 64, sc,
                                  h * W + xch * 128: h * W + (xch + 1) * 128],
                                gw_s[pc0:pc0 + 64, b, h, :],
                                start=(h == 0), stop=(h == 1),
                            )
                    ft = ft_pool.tile([128, 4 * 2 * S], bf16, tag="ft")
                    nc.scalar.copy(ft[:], ftp[:])

                    # O[j, n] = sum_x FT[x, j] * Gx[x, n]
                    op = pso_pool.tile([2 * S, 2 * S], f32, tag="op")
                    for xch in range(4):
                        nc.tensor.matmul(
                            op[:],
                            ft[:, xch * 64:(xch + 1) * 64],
                            gx_s[:, xch, b * 64: b * 64 + 64],
                            start=(xch == 0), stop=(xch == 3),
                        )
                    nc.vector.tensor_reduce(
                        acc[0:S, j:j + 1], op[0:S, 0:S],
                        axis=mybir.AxisListType.X, op=mybir.AluOpType.add,
                        apply_absolute_value=True,
                    )
                    nc.vector.tensor_reduce(
                        acc[S:2 * S, j:j + 1], op[S:2 * S, S:2 * S],
                        axis=mybir.AxisListType.X, op=mybir.AluOpType.add,
                        apply_absolute_value=True,
                    )

            nc.sync.dma_start(out=out.ap()[:, :], in_=acc[:])

    nc.compile()
    return nc


def _get_compiled():
    global _COMPILED
    if _COMPILED is None:
        _COMPILED = _build_nc()
    return _COMPILED


def _run_device(pred, target, landmarks, trace=False):
    from concourse import bass_utils

    pred = np.ascontiguousarray(np.asarray(pred, dtype=np.float32))
    target = np.ascontiguousarray(np.asarray(target, dtype=np.float32))
    landmarks = np.asarray(landmarks, dtype=np.float32)

    Gw, Gx, idx_tab = _build_host_tables(landmarks)
    gw_bf = Gw.astype(ml_dtypes.bfloat16)
    # (B, 512, 64) -> (128, 4, B, 64) bf16
    gx_packed = np.ascontiguousarray(
        Gx.reshape(B, 4, 128, 2 * S).transpose(2, 1, 0, 3)
    ).astype(ml_dtypes.bfloat16)

    in_maps = []
    for i in range(N_CORES):
        lo, hi = i * BPC, (i + 1) * BPC
        in_maps.append({
            "pred": pred[lo:hi],
            "target": target[lo:hi],
            "gw": gw_bf[lo:hi],
            "gx": np.ascontiguousarray(gx_packed[:, :, lo:hi]),
            "idx": np.ascontiguousarray(
                idx_tab[lo // 2: hi // 2].transpose(1, 0, 2).reshape(128, -1)),
        })

    nc = _get_compiled()
    res = bass_utils.run_bass_kernel_spmd(
        nc, in_maps, core_ids=list(range(N_CORES)), trace=trace
    )
    total = np.float64(0.0)
    for i in range(N_CORES):
        total += res.results[i]["o"].astype(np.float64).sum()
    n = B * C * S * S
    loss = total / n / 2.0
    return np.float32(loss), res


def kernel(pred, target, landmarks):
    loss, _ = _run_device(pred, target, landmarks, trace=False)
    return loss
